# revision 1
# baseline (speedup 1.0000x reference)
"""GQA attention kernel for 8 Trainium2 NeuronCores (v2).

Sharding: core c handles batch b = c//4, query rows [512*(c%4), 512*(c%4)+512).
Each core computes K/V for its batch's full (rolled) sequence, all 16 heads of
attention for its 512 query rows, and the final projection. No collectives.

Layouts (contraction/head dim on partitions):
  xfT [E,N] rolled so this core's q rows are cols 0:512; kn/qn [m, n|r];
  v [keys, g, d] with a ones column per group (softmax denominator rides the
  attn@v matmul for free).

Key structure vs v1:
  - every matmul operand is bf16 (4x faster than fp32 on the PE);
  - attn@v runs "flipped" (out = [q, d+1], free size 65) which halves its PE
    cost; the softmax division becomes a per-partition tensor_scalar that
    rides the mandatory PSUM->SBUF copy; the [q,d]->[m,q] layout fix for the
    projection is done by DMA XBAR transposes, not the PE;
  - rmsnorm+rope restructured: raw=(psum+bias) on DVE, squares and the two
    rope products on GpSimd, rotate-half via a p2 permutation matmul,
    sum-of-squares via a mask matmul packed 4 blocks per PSUM bank, one Sqrt
    per 4 blocks, norm scale broadcast via a mask matmul, applied in the
    final elementwise multiply;
  - one shared [128,1024] PSUM tag for projections, scores and proj
    accumulators keeps the 8-bank budget.
"""

import numpy as np

import concourse.bass as bass
import concourse.tile as tile
from concourse import bacc, mybir
from concourse import bass_utils

B, N, E = 2, 2048, 1024
H, KV, D = 16, 4, 64
R = 512            # query rows per core
EPS = 1e-6
F32 = mybir.dt.float32
F32R = mybir.dt.float32r
U32 = mybir.dt.uint32
BF16 = mybir.dt.bfloat16
AF = mybir.ActivationFunctionType
ALU = mybir.AluOpType

# head order: tile t holds (HEAD_ORDER[2t] at rows 0:64, HEAD_ORDER[2t+1] at 64:128)
HEAD_ORDER = [0, 4, 1, 5, 2, 6, 3, 7, 8, 12, 9, 13, 10, 14, 11, 15]


def _emit(tc, dr):
    nc = tc.nc
    with (
        tc.tile_pool(name="pers", bufs=1) as pers,
        tc.tile_pool(name="work", bufs=2) as wk,
        tc.tile_pool(name="wqs", bufs=2) as wqs,
        tc.tile_pool(name="ets", bufs=16) as ets,
        tc.tile_pool(name="outs", bufs=2) as outs,
        tc.tile_pool(name="big", bufs=3, space=bass.MemorySpace.PSUM) as big,
        tc.tile_pool(name="nrm", bufs=2, space=bass.MemorySpace.PSUM) as nrm,
    ):
        # ---------------- persistent tiles ----------------
        kt_til = [pers.tile([128, N], BF16, tag=f"kt{i}", name=f"ktt{i}")
                  for i in range(2)]
        qt_til = [pers.tile([128, 2, R], BF16, tag=f"qt{i}", name=f"qtt{i}")
                  for i in range(4)]
        vt_t = pers.tile([128, 16, 4, 65], BF16, tag="vt")  # v + ones col per g
        ot_t = pers.tile([128, 8, R], BF16, tag="ot")      # attn out (m, q)
        p2_t = pers.tile([128, 128], F32R, tag="p2")       # rotate-half perm
        bcm_t = pers.tile([128, 128], F32R, tag="bcm")     # bcast mask (x8 fold)
        smk_t = pers.tile([128, 2], F32R, tag="smk")       # 64-group col sums
        one_t = pers.tile([1, 128], BF16, tag="one")
        bq_t = pers.tile([128, 8], F32, tag="bq")
        bk_t = pers.tile([128, 2], F32, tag="bk")
        bv_t = pers.tile([1, 2, 512], BF16, tag="bv")      # per-pass tiled v bias
        bp_t = pers.tile([1, 2, 512], BF16, tag="bp")
        eps_t = pers.tile([128, 1], F32, tag="eps")

        xk_t = pers.tile([128, 8, N], BF16, tag="xk")
        wk_t = pers.tile([128, 8, 256], BF16, tag="wk")
        wv_t = pers.tile([128, 8, 256], BF16, tag="wv")
        ck_t = pers.tile([128, N], BF16, tag="ck")    # cos*w for K cols
        skp_t = pers.tile([128, N], BF16, tag="skp")  # permuted sign*sin*w for K
        cq_t = pers.tile([128, R], F32, tag="cq")
        sqp_t = pers.tile([128, R], F32, tag="sqp")
        kmag_t = pers.tile([2, 512], U32, tag="kmag")  # 0x5f3759df

        # startup-critical loads first: HWDGE dispatches in order
        xr = dr["xfT"].rearrange("(e p) n -> p e n", p=128)
        nc.sync.dma_start(out=wk_t, in_=dr["wkT"].rearrange("(e p) m -> p e m", p=128))
        for eh in range(2):
            nc.sync.dma_start(
                out=xk_t[:, 4 * eh:4 * eh + 4, 0:1024],
                in_=xr[:, 4 * eh:4 * eh + 4, 0:1024])
        nc.sync.dma_start(out=bk_t, in_=dr["bk"])
        nc.sync.dma_start(out=smk_t, in_=dr["summask"])
        nc.sync.dma_start(out=bcm_t, in_=dr["bcmask"])
        nc.sync.dma_start(out=p2_t, in_=dr["p2"])
        nc.sync.dma_start(out=ck_t, in_=dr["ckT"])
        nc.sync.dma_start(out=skp_t, in_=dr["skpT"])
        wq_c0 = wqs.tile([128, 8, 256], BF16, tag="wqc", name="wqc0")
        wqr = dr["wqT"].rearrange("(e p) m -> p e m", p=128)
        nc.sync.dma_start(out=wq_c0, in_=wqr[:, :, 0:256])
        nc.sync.dma_start(out=bq_t, in_=dr["bq"])
        nc.sync.dma_start(out=cq_t, in_=dr["cqT"])
        nc.sync.dma_start(out=sqp_t, in_=dr["sqpT"])
        for eh in range(2):
            nc.sync.dma_start(
                out=xk_t[:, 4 * eh:4 * eh + 4, 1024:2048],
                in_=xr[:, 4 * eh:4 * eh + 4, 1024:2048])
        nc.sync.dma_start(out=one_t, in_=dr["ones1"])
        nc.sync.dma_start(out=bv_t, in_=dr["bv"])
        nc.sync.dma_start(out=bp_t, in_=dr["bp"])
        nc.sync.dma_start(out=wv_t, in_=dr["wvT"].rearrange("(e p) m -> p e m", p=128))
        nc.vector.memset(vt_t[:, :, :, 64:65], 1.0)
        nc.vector.memset(kmag_t, 0x5F3759DF)
        nc.vector.memset(eps_t, 64.0 * EPS)

        # ---------- norm+rope ----------
        # rn = (psum+bias) * rsv, rsv = 1/sqrt(mean(raw^2)+eps) broadcast by a
        # mask matmul; rope: kn = rn*cos + P@(rn*sinp).  Startup units compute
        # rsv with Act Square+Sqrt (short chain, before any exp is emitted so
        # the activation table loads exactly twice); steady-state units use
        # the bit-trick inverse sqrt on DVE/Pool so exp owns the Act engine.
        def norm_rope(pr, bias_aps, cs_fn, sp_fn, out_fn, nm, fast):
            # raw = psum+bias; rope path (u, t2p, s) and rsqrt path (sq, pks,
            # prb) run in parallel; the norm scale is applied last:
            # kn = (rn*cos + P@(rn*sinp)) * prb  with prb = bcast(8/sqrt(..)).
            raw = wk.tile([128, 2, 512], F32, tag="raw", name="raw")
            for j in range(2):
                nc.vector.tensor_scalar_add(out=raw[:, j, :],
                                            in0=pr[:, 512 * j:512 * (j + 1)],
                                            scalar1=bias_aps[j])
            sq = wk.tile([128, 2, 512], F32R, tag="sqt", bufs=1, name="sq")
            if fast:
                nc.vector.tensor_mul(sq, raw, raw)
            else:
                for j in range(2):
                    nc.scalar.activation(out=sq[:, j, :],
                                         in_=pr[:, 512 * j:512 * (j + 1)],
                                         func=AF.Square, bias=bias_aps[j])
            u = wk.tile([128, 2, 512], F32R, tag="ut", name="u")
            t1 = wk.tile([128, 2, 512], F32, tag="t1t", bufs=1, name="t1")
            for j in range(2):
                nc.vector.tensor_mul(u[:, j, :], raw[:, j, :], sp_fn(j))
                nc.gpsimd.tensor_mul(t1[:, j, :], raw[:, j, :], cs_fn(j))
            pks = [None, None]
            for j in range(2):
                pks[j] = nrm.tile([128, 512], F32, tag="nrm",
                                  name=f"pks{nm}{j}")
                nc.tensor.matmul(pks[j][0:2, :], smk_t, sq[:, j, :],
                                 start=True, stop=True)
            rsvs = []
            for j in range(2):
                rsv = wk.tile([2, 512], F32R, tag="rsv", name=f"rsv{j}")
                if fast:
                    # bit-trick seed + one Newton step; eps dropped (the sum
                    # of 64 squares of projection outputs is never near 0)
                    sh = wk.tile([2, 512], U32, tag="sh", bufs=2, name="sh")
                    nc.vector.tensor_scalar(out=sh,
                                            in0=pks[j][0:2, :].bitcast(U32),
                                            scalar1=1, scalar2=None,
                                            op0=ALU.logical_shift_right)
                    y0b = wk.tile([2, 512], U32, tag="y0b", bufs=2, name="y0b")
                    nc.vector.tensor_tensor(out=y0b, in0=kmag_t, in1=sh,
                                            op=ALU.subtract)
                    y2 = wk.tile([2, 512], F32, tag="y2t", bufs=1, name="y2")
                    nc.vector.tensor_mul(y2, y0b.bitcast(F32),
                                         y0b.bitcast(F32))
                    nb = wk.tile([2, 512], F32, tag="nbt", bufs=2, name="nb")
                    nc.vector.scalar_tensor_tensor(out=nb, in0=pks[j][0:2, :],
                                                   scalar=-0.5, in1=y2,
                                                   op0=ALU.mult, op1=ALU.mult)
                    nc.vector.scalar_tensor_tensor(out=rsv, in0=nb, scalar=1.5,
                                                   in1=y0b.bitcast(F32),
                                                   op0=ALU.add, op1=ALU.mult)
                else:
                    sdk = wk.tile([2, 512], F32, tag="vv", bufs=2, name="sdk")
                    nc.scalar.activation(out=sdk, in_=pks[j][0:2, :],
                                         func=AF.Sqrt, bias=eps_t[0:2],
                                         scale=1.0)
                    with nc.allow_low_precision(reason="bf16-level norm"):
                        nc.vector.reciprocal(out=rsv, in_=sdk)
                rsvs.append(rsv)
            for j in range(2):
                t2p = nrm.tile([128, 512], F32, tag="nrm", name=f"t2p{nm}{j}")
                nc.tensor.matmul(t2p, p2_t, u[:, j, :], start=True, stop=True)
                s = wk.tile([128, 512], F32, tag="st", name="s")
                nc.vector.scalar_tensor_tensor(
                    out=s, in0=t2p, scalar=0.0, in1=t1[:, j, :],
                    op0=ALU.add, op1=ALU.add)
                prb = nrm.tile([128, 512], F32, tag="nrm", name=f"prb{nm}{j}")
                nc.tensor.matmul(prb, bcm_t[0:2, :], rsvs[j],
                                 start=True, stop=True)
                nc.vector.tensor_mul(out_fn(j), s, prb)

        # ---------------- stage-1 unit emitters ----------------
        def k_unit(kt, nbp, fast):
            pr = big.tile([128, 1024], F32, tag="big", name=f"pk{kt}{nbp}")
            for j in range(2):
                nb = 2 * nbp + j
                for e in range(8):
                    nc.tensor.matmul(pr[:, 512 * j:512 * (j + 1)],
                                     wk_t[:, e, 128 * kt:128 * (kt + 1)],
                                     xk_t[:, e, 512 * nb:512 * (nb + 1)],
                                     start=(e == 0), stop=(e == 7))
            norm_rope(
                pr, [bk_t[:, kt:kt + 1]] * 2,
                lambda j, nbp=nbp: ck_t[:, 1024 * nbp + 512 * j:
                                        1024 * nbp + 512 * (j + 1)],
                lambda j, nbp=nbp: skp_t[:, 1024 * nbp + 512 * j:
                                         1024 * nbp + 512 * (j + 1)],
                lambda j, kt=kt, nbp=nbp: kt_til[kt][:, 1024 * nbp + 512 * j:
                                                     1024 * nbp + 512 * (j + 1)],
                f"k{kt}{nbp}", fast)

        # V projection in two group-passes (gp=0: groups 0/1, gp=1: 2/3) so
        # attnv for head tiles 0-3 can start after pass A only.
        def v_unit(gp, half):
            pv = big.tile([128, 1024], F32, tag="big", name=f"pv{gp}{half}")
            for c in range(8):
                nch = 8 * half + c
                for e in range(8):
                    nc.tensor.matmul(pv[:, 128 * c:128 * (c + 1)],
                                     xk_t[:, e, 128 * nch:128 * (nch + 1)],
                                     wv_t[:, e, 128 * gp:128 * (gp + 1)],
                                     start=(e == 0), stop=False)
                nc.tensor.matmul(pv[:, 128 * c:128 * (c + 1)], one_t,
                                 bv_t[:, gp, 0:128], start=False, stop=True)
            nc.vector.tensor_copy(
                out=vt_t[:, 8 * half:8 * half + 8, 2 * gp:2 * gp + 2, 0:64],
                in_=pv.rearrange("p (c g x) -> p c g x", c=8, g=2))

        def q_unit(qp, fast):
            if qp == 0:
                wq_c = wq_c0
            else:
                wq_c = wqs.tile([128, 8, 256], BF16, tag="wqc", name=f"wqc{qp}")
                nc.sync.dma_start(out=wq_c,
                                  in_=wqr[:, :, 256 * qp:256 * (qp + 1)])
            pq = big.tile([128, 1024], F32, tag="big", name=f"pq{qp}")
            for j in range(2):
                for e in range(8):
                    nc.tensor.matmul(pq[:, 512 * j:512 * (j + 1)],
                                     wq_c[:, e, 128 * j:128 * (j + 1)],
                                     xk_t[:, e, 0:R],
                                     start=(e == 0), stop=(e == 7))
            norm_rope(
                pq,
                [bq_t[:, 2 * qp:2 * qp + 1], bq_t[:, 2 * qp + 1:2 * qp + 2]],
                lambda j: cq_t, lambda j: sqp_t,
                lambda j, qp=qp: qt_til[qp][:, j, :],
                f"q{qp}", fast)

        # ---------------- stage-2 unit emitters ----------------
        et_store = {}
        od_store = {}

        def score_unit(t, r01, fills=()):
            ktile = t // 4
            h = HEAD_ORDER[2 * t + r01]
            gq = h // 4
            prow = 64 * (gq % 2)
            assert gq // 2 == ktile and prow == 64 * r01
            qn_h = qt_til[t // 2][prow:prow + 64, t % 2, :]
            etl = []
            fi = 0
            for w in range(8):
                ps = big.tile([128, 1024], F32, tag="big", name=f"ps{t}{r01}{w}")
                for c in range(2):
                    nch = 2 * w + c
                    nc.tensor.matmul(
                        ps[:, 512 * c:512 * (c + 1)],
                        kt_til[ktile][prow:prow + 64, 128 * nch:128 * (nch + 1)],
                        qn_h, start=True, stop=True)
                et = ets.tile([128, 1024], BF16, tag="et", bufs=16,
                              name=f"et{t}{r01}{w}")
                etl.append(et)
                nc.scalar.activation(out=et, in_=ps, func=AF.Exp, scale=0.125)
                if w in (1, 3, 5) and fi < len(fills):
                    fills[fi]()
                    fi += 1
            et_store[(t, r01)] = etl
            for f in fills[fi:]:
                f()

        def attnv_unit(t, r01):
            h = HEAD_ORDER[2 * t + r01]
            gq = h // 4
            etl = et_store.pop((t, r01))
            if r01 == 0:
                od_store[t] = outs.tile([128, 4, 128], BF16, tag="od",
                                        name=f"od{t}")
            od = od_store[t]
            po = nrm.tile([128, 512], F32, tag="nrm", name=f"po{t}{r01}")
            for qc in range(4):
                for nch in range(16):
                    nc.tensor.matmul(
                        po[:, 65 * qc:65 * (qc + 1)],
                        etl[nch // 2][:, 512 * (nch % 2) + 128 * qc:
                                      512 * (nch % 2) + 128 * (qc + 1)],
                        vt_t[:, nch, gq, :],
                        start=(nch == 0), stop=(nch == 15))
            rcp = outs.tile([128, 4, 1], F32, tag="rcp", name=f"rcp{t}{r01}")
            for qc in range(4):
                nc.vector.reciprocal(out=rcp[:, qc, :],
                                     in_=po[:, 65 * qc + 64:65 * qc + 65])
                nc.vector.tensor_scalar_mul(
                    out=od[:, qc, 64 * r01:64 * r01 + 64],
                    in0=po[:, 65 * qc:65 * qc + 64], scalar1=rcp[:, qc, :])

        def transp_unit(t):
            od = od_store.pop(t)
            for qc in range(4):
                nc.sync.dma_start(out=ot_t[:, t, 128 * qc:128 * (qc + 1)],
                                  in_=od[:, qc, :], transpose=True)

        pjr = dr["pjT"].rearrange("(m p) e -> p m e", p=128)
        pjc_store = {}

        def pjc_unit(half, mp, tag="pjc"):
            t_ = wqs.tile([128, 2, 512], BF16, tag=tag, bufs=2,
                          name=f"pjc{half}{mp}")
            nc.sync.dma_start(
                out=t_, in_=pjr[:, 2 * mp:2 * mp + 2,
                                512 * half:512 * (half + 1)])
            pjc_store[(half, mp)] = t_

        foA = {}

        def proj_a(half):
            pf = [big.tile([128, 1024], F32, tag="big", name=f"pfa{half}{p}")
                  for p in range(2)]
            for mt in range(6):
                if (half, mt // 2) not in pjc_store:
                    pjc_unit(half, mt // 2)
                pj_c = pjc_store[(half, mt // 2)]
                for rc in range(4):
                    nc.tensor.matmul(pf[rc // 2][:, 512 * (rc % 2):
                                                 512 * (rc % 2 + 1)],
                                     ot_t[:, mt, 128 * rc:128 * (rc + 1)],
                                     pj_c[:, mt % 2, :],
                                     start=(mt == 0), stop=(mt == 5))
                if mt % 2 == 1:
                    pjc_store.pop((half, mt // 2))
            for p in range(2):
                fa = outs.tile([128, 1024], BF16, tag="foa", bufs=4,
                               name=f"foa{half}{p}")
                nc.vector.tensor_copy(out=fa, in_=pf[p])
                foA[(half, p)] = fa

        def proj_tail():
            for half in range(2):
                pf = [big.tile([128, 1024], F32, tag="big", name=f"pf{half}{p}")
                      for p in range(2)]
                for mt in range(6, 8):
                    if (half, mt // 2) not in pjc_store:
                        pjc_unit(half, mt // 2)
                    pj_c = pjc_store[(half, mt // 2)]
                    for rc in range(4):
                        nc.tensor.matmul(pf[rc // 2][:, 512 * (rc % 2):
                                                     512 * (rc % 2 + 1)],
                                         ot_t[:, mt, 128 * rc:128 * (rc + 1)],
                                         pj_c[:, mt % 2, :],
                                         start=(mt == 6), stop=False)
                    if mt % 2 == 1:
                        pjc_store.pop((half, mt // 2))
                for rc in range(4):
                    nc.tensor.matmul(pf[rc // 2][:, 512 * (rc % 2):
                                                 512 * (rc % 2 + 1)],
                                     one_t, bp_t[:, half, :],
                                     start=False, stop=True)
                for rc in range(4):
                    fo = outs.tile([128, 512], F32, tag="fo", bufs=4,
                                   name=f"fo{half}{rc}")
                    nc.vector.scalar_tensor_tensor(
                        out=fo, in0=pf[rc // 2][:, 512 * (rc % 2):
                                                512 * (rc % 2 + 1)],
                        scalar=0.0,
                        in1=foA[(half, rc // 2)][:, 512 * (rc % 2):
                                                 512 * (rc % 2 + 1)],
                        op0=ALU.add, op1=ALU.add)
                    nc.sync.dma_start(
                        out=dr["out"][128 * rc:128 * (rc + 1),
                                      512 * half:512 * (half + 1)],
                        in_=fo)

        # ================= schedule =================
        # Emission order defines dataflow: score(t) needs kt(ktile) + qt
        # tile t; attnv for tiles 0-3 needs V pass A (+ pass B for 4-7);
        # q_unit(qp) makes qt tiles 2qp/2qp+1.  Startup units use the
        # short-latency Act-based norm; filler units use the DVE/Pool
        # inverse sqrt so the Act engine streams exp uninterrupted.
        k_unit(0, 0, fast=False)
        k_unit(0, 1, fast=False)
        q_unit(0, fast=False)
        k_unit(1, 0, fast=False)
        k_unit(1, 1, fast=False)
        score_unit(0, 0, (lambda: v_unit(0, 0), lambda: v_unit(0, 1)))
        score_unit(0, 1, (lambda: q_unit(1, True), lambda: q_unit(2, True)))
        attnv_unit(0, 0)
        score_unit(1, 0, (lambda: q_unit(3, True), lambda: v_unit(1, 0)))
        attnv_unit(0, 1)
        transp_unit(0)
        score_unit(1, 1, (lambda: v_unit(1, 1),))
        attnv_unit(1, 0)
        score_unit(2, 0)
        attnv_unit(1, 1)
        transp_unit(1)
        score_unit(2, 1)
        attnv_unit(2, 0)
        score_unit(3, 0)
        attnv_unit(2, 1)
        transp_unit(2)
        score_unit(3, 1)
        attnv_unit(3, 0)
        score_unit(4, 0)
        attnv_unit(3, 1)
        transp_unit(3)
        for t in range(4, 8):
            fills = ()
            if t == 5:
                fills = (lambda: pjc_unit(0, 0, "pjh0"),)
            elif t == 6:
                fills = (lambda: pjc_unit(1, 0, "pjh0"),)
            score_unit(t, 1, fills)
            attnv_unit(t, 0)
            if t < 7:
                score_unit(t + 1, 0)
            attnv_unit(t, 1)
            transp_unit(t)
            if t == 5:
                proj_a(0)
            elif t == 6:
                proj_a(1)

        # ================= stage 3: output projection =================
        # pj streams as [128, 2, 512] 2-chunk tiles (3 bufs); the first tiles
        # are prefetched as fills inside the last heads so the tail is not
        # DMA-latency-bound.
        proj_tail()

_CACHE = {}


def _get_nc():
    if "nc" in _CACHE:
        return _CACHE["nc"]
    nc = bacc.Bacc("TRN2", target_bir_lowering=False, debug=False,
                   enable_asserts=False, num_devices=8)
    bf_shapes = {
        "xfT": (E, N), "wqT": (E, E), "wkT": (E, 256), "wvT": (E, 256),
        "pjT": (E, E), "ones1": (1, 128), "bv": (1, 2, 512), "bp": (1, 2, 512),
    }
    dr = {k: nc.dram_tensor(k, list(v), BF16, kind="ExternalInput").ap()
          for k, v in bf_shapes.items()}
    bf_shapes2 = {"ckT": (128, N), "skpT": (128, N)}
    for k, v in bf_shapes2.items():
        dr[k] = nc.dram_tensor(k, list(v), BF16, kind="ExternalInput").ap()
    for k, v in {"cqT": (128, R),
                 "sqpT": (128, R), "bq": (128, 8), "bk": (128, 2)}.items():
        dr[k] = nc.dram_tensor(k, list(v), F32, kind="ExternalInput").ap()
    for k, v in {"p2": (128, 128), "bcmask": (128, 128),
                 "summask": (128, 2)}.items():
        dr[k] = nc.dram_tensor(k, list(v), F32R, kind="ExternalInput").ap()
    dr["out"] = nc.dram_tensor("out", [R, E], F32, kind="ExternalOutput").ap()
    with tile.TileContext(nc) as tc:
        _emit(tc, dr)
    nc.compile()
    _CACHE["nc"] = nc
    return nc


def _host_prep(inputs):
    f = np.float32
    import ml_dtypes
    bf = ml_dtypes.bfloat16
    x = np.asarray(inputs["x"], f)
    sin = np.asarray(inputs["sin"], f)
    cos = np.asarray(inputs["cos"], f)
    qn_w = np.asarray(inputs["qn_w"], f)
    kn_w = np.asarray(inputs["kn_w"], f)
    d = np.arange(D)
    sw = d ^ 32
    sign = np.where(d < 32, -1.0, 1.0).astype(f)
    # cos tiles [64, N] rows indexed by d; w folded
    cq64 = (cos * qn_w).T.astype(f)
    ck64 = (cos * kn_w).T.astype(f)
    # permuted sin: sp[e, n] = -sign[e] * w[e] * sin[n, e^32]
    sq64p = (sin.T[sw, :] * (-sign * qn_w)[:, None]).astype(f)
    sk64p = (sin.T[sw, :] * (-sign * kn_w)[:, None]).astype(f)
    cq128 = np.tile(cq64, (2, 1))
    sq128p = np.tile(sq64p, (2, 1))
    ck128 = np.tile(ck64, (2, 1))
    sk128p = np.tile(sk64p, (2, 1))
    p2 = np.zeros((128, 128), f)
    i = np.arange(128)
    p2[i, (i // 64) * 64 + ((i % 64) ^ 32)] = 1.0
    bcm2 = np.zeros((2, 128), f)
    bcm2[0, 0:64] = 1.0
    bcm2[1, 64:128] = 1.0
    bcm128 = np.zeros((128, 128), f)
    bcm128[0:2, :] = 8.0 * bcm2
    smk = np.ascontiguousarray(bcm2.T)
    ones1 = np.ones((1, 128), f)
    # head permutation: new m index -> old m index
    perm = np.concatenate([np.arange(64 * h, 64 * h + 64) for h in HEAD_ORDER])
    wqT = np.asarray(inputs["wq_w"], f).T   # [e, m]
    pjT = np.asarray(inputs["proj_w"], f).T  # [m, mo]
    bq = np.asarray(inputs["wq_b"], f)
    com = {
        "wqT": np.ascontiguousarray(wqT[:, perm]).astype(bf),
        "wkT": np.ascontiguousarray(np.asarray(inputs["wk_w"], f).T).astype(bf),
        "wvT": np.ascontiguousarray(np.asarray(inputs["wv_w"], f).T).astype(bf),
        "pjT": np.ascontiguousarray(pjT[perm, :]).astype(bf),
        "p2": p2, "bcmask": bcm128, "summask": smk,
        "ones1": ones1.astype(bf),
        "bq": np.ascontiguousarray(bq[perm].reshape(8, 128).T),
        "bk": np.ascontiguousarray(np.asarray(inputs["wk_b"], f).reshape(2, 128).T),
        "bv": np.stack([np.tile(np.asarray(inputs["wv_b"], f)[128 * gp:128 * (gp + 1)], 4)
                        for gp in range(2)])[None].astype(bf),
        "bp": np.asarray(inputs["proj_b"], f).reshape(1, 2, 512).astype(bf),
    }
    in_maps = []
    for c in range(8):
        b, ch = c // 4, c % 4
        roff = R * ch
        m = dict(com)
        m["xfT"] = np.ascontiguousarray(np.roll(x[b].T, -roff, axis=1)).astype(bf)
        m["ckT"] = np.ascontiguousarray(np.roll(ck128, -roff, axis=1)).astype(bf)
        m["skpT"] = np.ascontiguousarray(np.roll(sk128p, -roff, axis=1)).astype(bf)
        m["cqT"] = np.ascontiguousarray(cq128[:, roff:roff + R])
        m["sqpT"] = np.ascontiguousarray(sq128p[:, roff:roff + R])
        in_maps.append(m)
    return in_maps


def kernel(**inputs):
    nc = _get_nc()
    in_maps = _host_prep(inputs)
    res = bass_utils.run_bass_kernel_spmd(nc, in_maps, core_ids=list(range(8)))
    out = np.empty((B, N, E), np.float32)
    for c in range(8):
        b, ch = c // 4, c % 4
        out[b, R * ch:R * (ch + 1), :] = res.results[c]["out"]
    return out



# revision 19
# speedup vs baseline: 1.2049x; 1.2049x over previous
"""GQA attention kernel for 8 Trainium2 NeuronCores (v3).

Sharding: core c handles batch b = c//4, query rows [512*(c%4), 512*(c%4)+512).
Each core computes K/V for its batch's full (rolled) sequence, all 16 heads of
attention for its 512 query rows, and the final projection. No collectives.

v3 is a scheduling rewrite of v2 driven by TimelineSim engine occupancy:
the Act engine's exp stream (16 units x 8 x [128,1024] ~ 133us) is the hard
floor, and the PE's 154us of matmuls must hide almost entirely under it.

  - QKV/norm/proj work is emitted as <=1us "fill granules" interleaved into
    the score units' w-slots so the PE never runs far ahead or behind the
    exp stream (ps pool double-buffering absorbs one slot of jitter);
  - PSUM re-layout: ps 2x[128,1024] + aux 1x[128,512] (qkv/proj psum) +
    nrm 1x[128,512] (t2p/prb) + po 1x[128,4,65] + pkb 1x[8,512] = 16KB;
  - rsqrt chains are batched: up to 4 subs' sum-of-squares accumulate into
    one [8,512] PSUM tile via zero-padded mask stationaries, so one 5-op
    DVE Newton chain serves 4 norm blocks;
  - v bias folded into the output bias (softmax rows sum to 1 =>
    attn@(v+b) = attn@v + b), proj bias folded into a broadcast SBUF tile
    added during the proj_a PSUM->SBUF copy: zero bias matmuls;
  - startup DMAs split into 512-col chunks ordered along the critical path
    (wk, xk0 -> first k matmul at ~4.5us, first exp at ~13us);
  - projection runs as per-(half,rc) chunks: mt0-5 as fills in units 6-7
    (+ bias via bpb), tail = 2 matmuls per chunk through the freed ps ring.
"""

import numpy as np

import concourse.bass as bass
import concourse.tile as tile
from concourse import bacc, mybir
from concourse import bass_utils

B, N, E = 2, 2048, 1024
H, KV, D = 16, 4, 64
R = 512            # query rows per core
EPS = 1e-6
F32 = mybir.dt.float32
F32R = mybir.dt.float32r
U32 = mybir.dt.uint32
BF16 = mybir.dt.bfloat16
AF = mybir.ActivationFunctionType
ALU = mybir.AluOpType

# head order: tile t holds (HEAD_ORDER[2t] at rows 0:64, HEAD_ORDER[2t+1] at 64:128)
HEAD_ORDER = [0, 4, 1, 5, 2, 6, 3, 7, 8, 12, 9, 13, 10, 14, 11, 15]


def _emit(tc, dr):
    nc = tc.nc
    with (
        tc.tile_pool(name="pers", bufs=1) as pers,
        tc.tile_pool(name="work", bufs=2) as wk,
        tc.tile_pool(name="wqs", bufs=2) as wqs,
        tc.tile_pool(name="ets", bufs=16) as ets,
        tc.tile_pool(name="outs", bufs=2) as outs,
        tc.tile_pool(name="psp", bufs=2, space=bass.MemorySpace.PSUM) as psp,
        tc.tile_pool(name="auxp", bufs=1, space=bass.MemorySpace.PSUM) as auxp,
        tc.tile_pool(name="nrmp", bufs=1, space=bass.MemorySpace.PSUM) as nrmp,
        tc.tile_pool(name="pop", bufs=1, space=bass.MemorySpace.PSUM) as pop,
        tc.tile_pool(name="pkbp", bufs=1, space=bass.MemorySpace.PSUM) as pkbp,
    ):
        # ---------------- persistent tiles ----------------
        kt_til = [pers.tile([128, N], BF16, tag=f"kt{i}", name=f"ktt{i}")
                  for i in range(2)]
        qt_til = [pers.tile([128, 2, R], BF16, tag=f"qt{i}", name=f"qtt{i}")
                  for i in range(4)]
        vt_t = pers.tile([128, 16, 4, 65], BF16, tag="vt")  # v + ones col per g
        ot_t = pers.tile([128, 8, R], BF16, tag="ot")      # attn out (m, q)
        pj_t = pers.tile([128, 8, 1024], BF16, tag="pj")   # proj weights
        p2_t = pers.tile([128, 128], F32R, tag="p2")       # rotate-half perm
        bcm_t = pers.tile([128, 128], F32R, tag="bcm")     # bcast mask (x8 fold)
        smk_t = pers.tile([128, 3, 66], F32R, tag="smk")   # col-sum masks, 3 pads
        bq_t = pers.tile([128, 8], F32, tag="bq")
        bk_t = pers.tile([128, 2], F32, tag="bk")
        bpb_t = pers.tile([128, 2, 512], BF16, tag="bpb")  # proj+v bias bcast
        eps_t = pers.tile([128, 1], F32, tag="eps")
        kmag_t = pers.tile([128, 512], U32, tag="kmag")    # 0x5f3759df

        xk_t = pers.tile([128, 8, N], BF16, tag="xk")
        wk_t = pers.tile([128, 8, 256], BF16, tag="wk")
        wv_t = pers.tile([128, 8, 256], BF16, tag="wv")
        ck_t = pers.tile([128, N], BF16, tag="ck")    # cos*w for K cols
        skp_t = pers.tile([128, N], BF16, tag="skp")  # permuted sign*sin*w for K
        cq_t = pers.tile([128, R], F32, tag="cq")
        sqp_t = pers.tile([128, R], F32, tag="sqp")

        nc.vector.memset(vt_t[:, :, :, 64:65], 1.0)
        nc.vector.memset(kmag_t, 0x5F3759DF)
        nc.vector.memset(eps_t, 64.0 * EPS)

        # ---------- startup DMAs, ordered along the critical path ----------
        xr = dr["xfT"].rearrange("(e p) n -> p e n", p=128)
        nc.sync.dma_start(out=wk_t, in_=dr["wkT"].rearrange("(e p) m -> p e m", p=128))
        nc.sync.dma_start(out=xk_t[:, :, 0:512], in_=xr[:, :, 0:512])
        nc.sync.dma_start(out=skp_t[:, 0:1024], in_=dr["skpT"][:, 0:1024])
        nc.sync.dma_start(out=ck_t[:, 0:1024], in_=dr["ckT"][:, 0:1024])
        nc.sync.dma_start(out=bk_t, in_=dr["bk"])
        nc.sync.dma_start(out=smk_t, in_=dr["summask"])
        nc.sync.dma_start(out=p2_t, in_=dr["p2"])
        nc.sync.dma_start(out=xk_t[:, :, 512:1024], in_=xr[:, :, 512:1024])
        wq_c0 = wqs.tile([128, 8, 256], BF16, tag="wqc", name="wqc0")
        wqr = dr["wqT"].rearrange("(e p) m -> p e m", p=128)
        nc.sync.dma_start(out=wq_c0, in_=wqr[:, :, 0:256])
        nc.sync.dma_start(out=bq_t, in_=dr["bq"])
        nc.sync.dma_start(out=cq_t, in_=dr["cqT"])
        nc.sync.dma_start(out=sqp_t, in_=dr["sqpT"])
        nc.sync.dma_start(out=bcm_t, in_=dr["bcmask"])
        nc.sync.dma_start(out=xk_t[:, :, 1024:1536], in_=xr[:, :, 1024:1536])
        nc.sync.dma_start(out=xk_t[:, :, 1536:2048], in_=xr[:, :, 1536:2048])
        nc.sync.dma_start(out=wv_t, in_=dr["wvT"].rearrange("(e p) m -> p e m", p=128))
        nc.sync.dma_start(out=ck_t[:, 1024:2048], in_=dr["ckT"][:, 1024:2048])
        nc.sync.dma_start(out=skp_t[:, 1024:2048], in_=dr["skpT"][:, 1024:2048])
        nc.sync.dma_start(out=bpb_t, in_=dr["bpb"])

        # ---------------- norm machinery ----------------
        # Per 128-row m-block x 512-token "sub": raw = psum+bias on DVE;
        # squares (gpsimd fast / Act Square slow); sum-of-squares via a
        # zero-padded mask matmul accumulating batch row-pair 2i; rope
        # products u (gpsimd) and t1 (gpsimd); rsqrt for a whole batch in
        # one 5-op DVE Newton chain ([8,512]: free size stays 512); then
        # t2p = P@u, s = t2p+t1, prb = bcast(rsv), out = s*prb.
        class Batch:
            def __init__(self, nm, n_subs, slow):
                self.nm = nm
                self.n = n_subs
                self.slow = slow
                self.next_i = 0
                self.tile = None
                self.rsv = None

        st_u = {}
        st_t1 = {}
        st_s = {}

        def norm_front(key, pr, bias_ap, cs_ap, sp_ap, batch):
            raw = wk.tile([128, 512], F32, tag="raw", bufs=3, name=f"raw{key}")
            nc.vector.tensor_scalar_add(out=raw, in0=pr, scalar1=bias_ap)
            sq = wk.tile([128, 512], F32R, tag="sq", bufs=2, name=f"sq{key}")
            if batch.slow:
                nc.scalar.activation(out=sq, in_=pr, func=AF.Square,
                                     bias=bias_ap)
            else:
                nc.gpsimd.tensor_mul(sq, raw, raw)
            i = batch.next_i
            batch.next_i += 1
            if i == 0:
                batch.tile = pkbp.tile([66, 512], F32, tag="pkb",
                                       name=f"pkb{batch.nm}")
            nc.tensor.matmul(batch.tile, smk_t[:, i, :], sq,
                             start=(i == 0), stop=(i == batch.n - 1))
            u = wk.tile([128, 512], F32R, tag="u", bufs=3, name=f"u{key}")
            nc.gpsimd.tensor_mul(u, raw, sp_ap)
            t1 = wk.tile([128, 512], F32, tag="t1", bufs=3, name=f"t1{key}")
            nc.gpsimd.tensor_mul(t1, raw, cs_ap)
            st_u[key] = u
            st_t1[key] = t1
            return i

        def chain(batch):
            pk = batch.tile
            r = 32 * (batch.n - 1) + 2
            rv = wk.tile([66, 512], F32R, tag="rv", bufs=2,
                         name=f"rv{batch.nm}")
            if batch.slow:
                sdk = wk.tile([66, 512], F32, tag="sdk", bufs=1,
                              name=f"sdk{batch.nm}")
                nc.scalar.activation(out=sdk[0:r], in_=pk[0:r], func=AF.Sqrt,
                                     bias=eps_t[0:r], scale=1.0)
                with nc.allow_low_precision(reason="bf16-level norm"):
                    nc.vector.reciprocal(out=rv[0:r], in_=sdk[0:r])
            else:
                sh = wk.tile([66, 512], U32, tag="sh", bufs=1,
                             name=f"sh{batch.nm}")
                nc.vector.tensor_scalar(out=sh[0:r], in0=pk[0:r].bitcast(U32),
                                        scalar1=1, scalar2=None,
                                        op0=ALU.logical_shift_right)
                y0 = wk.tile([66, 512], U32, tag="y0", bufs=1,
                             name=f"y0{batch.nm}")
                nc.vector.tensor_tensor(out=y0[0:r], in0=kmag_t[0:r],
                                        in1=sh[0:r], op=ALU.subtract)
                y2 = wk.tile([66, 512], F32, tag="y2", bufs=1,
                             name=f"y2{batch.nm}")
                nc.vector.tensor_mul(y2[0:r], y0[0:r].bitcast(F32),
                                     y0[0:r].bitcast(F32))
                nb = wk.tile([66, 512], F32, tag="nb", bufs=1,
                             name=f"nb{batch.nm}")
                nc.vector.scalar_tensor_tensor(out=nb[0:r], in0=pk[0:r],
                                               scalar=-0.5, in1=y2[0:r],
                                               op0=ALU.mult, op1=ALU.mult)
                nc.vector.scalar_tensor_tensor(out=rv[0:r], in0=nb[0:r],
                                               scalar=1.5,
                                               in1=y0[0:r].bitcast(F32),
                                               op0=ALU.add, op1=ALU.mult)
            batch.rsv = rv

        def norm_f2(key):
            u = st_u.pop(key)
            t2p = nrmp.tile([128, 512], F32, tag="nrm", name=f"t2p{key}")
            nc.tensor.matmul(t2p, p2_t, u, start=True, stop=True)
            s = wk.tile([128, 512], F32, tag="s", bufs=2, name=f"s{key}")
            nc.vector.scalar_tensor_tensor(out=s, in0=t2p, scalar=0.0,
                                           in1=st_t1.pop(key),
                                           op0=ALU.add, op1=ALU.add)
            st_s[key] = s

        def norm_f3(key, batch, i, out_ap):
            prb = nrmp.tile([128, 512], F32, tag="nrm", name=f"prb{key}")
            nc.tensor.matmul(prb, bcm_t[32 * i:32 * i + 2, :],
                             batch.rsv[32 * i:32 * i + 2],
                             start=True, stop=True)
            nc.vector.tensor_mul(out_ap, st_s.pop(key), prb)

        # ---------------- k / q / v sub emitters ----------------
        aux_store = {}
        sub_meta = {}

        def k_mm(kt, nb, part):
            key = f"k{kt}{nb}"
            if part == 0:
                aux_store[key] = auxp.tile([128, 512], F32, tag="aux",
                                           name=f"pk{key}")
            pr = aux_store[key]
            for e in range(4 * part, 4 * part + 4):
                nc.tensor.matmul(pr, wk_t[:, e, 128 * kt:128 * (kt + 1)],
                                 xk_t[:, e, 512 * nb:512 * (nb + 1)],
                                 start=(e == 0), stop=(e == 7))

        def k_front(kt, nb, batch):
            key = f"k{kt}{nb}"
            i = norm_front(key, aux_store.pop(key), bk_t[:, kt:kt + 1],
                           ck_t[:, 512 * nb:512 * (nb + 1)],
                           skp_t[:, 512 * nb:512 * (nb + 1)], batch)
            sub_meta[key] = (batch, i)

        def k_back2(kt, nb):
            norm_f2(f"k{kt}{nb}")

        def k_back3(kt, nb):
            key = f"k{kt}{nb}"
            batch, i = sub_meta.pop(key)
            norm_f3(key, batch, i,
                    kt_til[kt][:, 512 * nb:512 * (nb + 1)])

        wq_store = {0: wq_c0}

        def wq_dma(qp):
            wq_c = wqs.tile([128, 8, 256], BF16, tag="wqc", name=f"wqc{qp}")
            nc.sync.dma_start(out=wq_c, in_=wqr[:, :, 256 * qp:256 * (qp + 1)])
            wq_store[qp] = wq_c

        def q_mm(qp, j, part):
            key = f"q{qp}{j}"
            if part == 0:
                aux_store[key] = auxp.tile([128, 512], F32, tag="aux",
                                           name=f"pq{key}")
            pr = aux_store[key]
            wq_c = wq_store[qp]
            for e in range(4 * part, 4 * part + 4):
                nc.tensor.matmul(pr, wq_c[:, e, 128 * j:128 * (j + 1)],
                                 xk_t[:, e, 0:R],
                                 start=(e == 0), stop=(e == 7))

        def q_front(qp, j, batch):
            key = f"q{qp}{j}"
            i = norm_front(key, aux_store.pop(key),
                           bq_t[:, 2 * qp + j:2 * qp + j + 1],
                           cq_t, sqp_t, batch)
            sub_meta[key] = (batch, i)

        def q_back2(qp, j):
            norm_f2(f"q{qp}{j}")

        def q_back3(qp, j):
            key = f"q{qp}{j}"
            batch, i = sub_meta.pop(key)
            norm_f3(key, batch, i, qt_til[qp][:, j, :])

        def v_mm(gp, q4, cpair):
            key = f"v{gp}{q4}"
            if cpair == 0:
                aux_store[key] = auxp.tile([128, 512], F32, tag="aux",
                                           name=f"pv{key}")
            pv = aux_store[key]
            for c in range(2 * cpair, 2 * cpair + 2):
                nch = 4 * q4 + c
                for e in range(8):
                    nc.tensor.matmul(pv[:, 128 * c:128 * (c + 1)],
                                     xk_t[:, e, 128 * nch:128 * (nch + 1)],
                                     wv_t[:, e, 128 * gp:128 * (gp + 1)],
                                     start=(e == 0), stop=(e == 7))
            if cpair == 1:
                pv = aux_store.pop(key)
                nc.vector.tensor_copy(
                    out=vt_t[:, 4 * q4:4 * q4 + 4, 2 * gp:2 * gp + 2, 0:64],
                    in_=pv.rearrange("p (c g x) -> p c g x", c=4, g=2))

        # ---------------- stage-2 unit emitters ----------------
        et_store = {}
        od_store = {}

        def score_unit(t, r01, fills=()):
            ktile = t // 4
            h = HEAD_ORDER[2 * t + r01]
            gq = h // 4
            prow = 64 * (gq % 2)
            assert gq // 2 == ktile and prow == 64 * r01
            qn_h = qt_til[t // 2][prow:prow + 64, t % 2, :]
            etl = []
            fi = 0
            for w in range(8):
                ps = psp.tile([128, 1024], F32, tag="ps", name=f"ps{t}{r01}{w}")
                for c in range(2):
                    nch = 2 * w + c
                    nc.tensor.matmul(
                        ps[:, 512 * c:512 * (c + 1)],
                        kt_til[ktile][prow:prow + 64, 128 * nch:128 * (nch + 1)],
                        qn_h, start=True, stop=True)
                et = ets.tile([128, 1024], BF16, tag="et", bufs=16,
                              name=f"et{t}{r01}{w}")
                etl.append(et)
                nc.scalar.activation(out=et, in_=ps, func=AF.Exp, scale=0.125)
                if fi < len(fills):
                    fills[fi]()
                    fi += 1
            et_store[(t, r01)] = etl
            for f in fills[fi:]:
                f()

        def attnv_unit(t, r01):
            h = HEAD_ORDER[2 * t + r01]
            gq = h // 4
            etl = et_store.pop((t, r01))
            if r01 == 0:
                od_store[t] = outs.tile([128, 4, 128], BF16, tag="od",
                                        name=f"od{t}")
            od = od_store[t]
            po = pop.tile([128, 4, 65], F32, tag="po", name=f"po{t}{r01}")
            for qc in range(4):
                for nch in range(16):
                    nc.tensor.matmul(
                        po[:, qc, :],
                        etl[nch // 2][:, 512 * (nch % 2) + 128 * qc:
                                      512 * (nch % 2) + 128 * (qc + 1)],
                        vt_t[:, nch, gq, :],
                        start=(nch == 0), stop=(nch == 15))
            rcp = outs.tile([128, 4, 1], F32, tag="rcp", name=f"rcp{t}{r01}")
            for qc in range(4):
                nc.vector.reciprocal(out=rcp[:, qc, :],
                                     in_=po[:, qc, 64:65])
                nc.vector.tensor_scalar_mul(
                    out=od[:, qc, 64 * r01:64 * r01 + 64],
                    in0=po[:, qc, 0:64], scalar1=rcp[:, qc, :])

        def transp_unit(t):
            od = od_store.pop(t)
            for qc in range(4):
                nc.sync.dma_start(out=ot_t[:, t, 128 * qc:128 * (qc + 1)],
                                  in_=od[:, qc, :], transpose=True)

        # ---------------- projection ----------------
        pjr = dr["pjT"].rearrange("(m p) e -> p m e", p=128)

        def pj_dma(mp):
            nc.sync.dma_start(out=pj_t[:, 2 * mp:2 * mp + 2, :],
                              in_=pjr[:, 2 * mp:2 * mp + 2, :])

        pf_store = {}
        foA = {}

        def pa_mm(half, rc, g):
            # accumulate mt 3g..3g+2 of proj for output block (half, rc)
            if g == 0:
                pf_store[(half, rc)] = auxp.tile([128, 512], F32, tag="aux",
                                                 name=f"pf{half}{rc}")
            pf = pf_store[(half, rc)]
            for mt in range(3 * g, 3 * g + 3):
                nc.tensor.matmul(pf, ot_t[:, mt, 128 * rc:128 * (rc + 1)],
                                 pj_t[:, mt, 512 * half:512 * (half + 1)],
                                 start=(mt == 0), stop=(mt == 5))
            if g == 1:
                pf = pf_store.pop((half, rc))
                fa = outs.tile([128, 512], BF16, tag="foa", bufs=8,
                               name=f"foa{half}{rc}")
                nc.vector.tensor_tensor(out=fa, in0=pf,
                                        in1=bpb_t[:, half, :], op=ALU.add)
                foA[(half, rc)] = fa

        def pt_unit(half, rc):
            pf2 = psp.tile([128, 512], F32, tag="ps", name=f"pt{half}{rc}")
            for mt in (6, 7):
                nc.tensor.matmul(pf2, ot_t[:, mt, 128 * rc:128 * (rc + 1)],
                                 pj_t[:, mt, 512 * half:512 * (half + 1)],
                                 start=(mt == 6), stop=(mt == 7))
            fo = outs.tile([128, 512], F32, tag="fo", bufs=4,
                           name=f"fo{half}{rc}")
            nc.vector.scalar_tensor_tensor(
                out=fo, in0=pf2, scalar=0.0, in1=foA.pop((half, rc)),
                op0=ALU.add, op1=ALU.add)
            nc.sync.dma_start(
                out=dr["out"][128 * rc:128 * (rc + 1),
                              512 * half:512 * (half + 1)],
                in_=fo)

        # ================= schedule =================
        bS = Batch("S", 3, slow=True)      # k00, k01, q00 (Act sqrt prologue)
        bF = Batch("F", 3, slow=False)     # k02, k03, q01
        bK1a = Batch("K1a", 2, slow=False)  # k10, k11
        bK1b = Batch("K1b", 2, slow=False)  # k12, k13
        bQ1 = Batch("Q1", 2, slow=False)   # q10, q11
        bQ2 = Batch("Q2", 2, slow=False)   # q20, q21
        bQ3 = Batch("Q3", 2, slow=False)   # q30, q31

        # ---- prologue: minimal path to the first exp ----
        k_mm(0, 0, 0); k_mm(0, 0, 1); k_front(0, 0, bS)
        k_mm(0, 1, 0); k_mm(0, 1, 1); k_front(0, 1, bS)
        q_mm(0, 0, 0); q_mm(0, 0, 1); q_front(0, 0, bS)
        chain(bS)
        k_back2(0, 0); k_back3(0, 0)
        q_back2(0, 0); q_back3(0, 0)
        k_back2(0, 1); k_back3(0, 1)
        # k02/k03/q01 are DMA-gated (xk chunks 2,3); emit before the first
        # score unit (its w>=4 matmuls read their kt output regions) — the
        # scheduler still executes them as the data lands.
        k_mm(0, 2, 0); k_mm(0, 2, 1); k_front(0, 2, bF)
        k_mm(0, 3, 0); k_mm(0, 3, 1); k_front(0, 3, bF)
        q_mm(0, 1, 0); q_mm(0, 1, 1); q_front(0, 1, bF)
        chain(bF)
        k_back2(0, 2); k_back3(0, 2)
        k_back2(0, 3); k_back3(0, 3)
        q_back2(0, 1); q_back3(0, 1)

        score_unit(0, 0, (
            lambda: wq_dma(1),
            lambda: v_mm(0, 0, 0),
            lambda: v_mm(0, 0, 1),
            lambda: v_mm(0, 1, 0),
            lambda: v_mm(0, 1, 1),
        ))
        score_unit(0, 1, (
            lambda: v_mm(0, 2, 0),
            lambda: v_mm(0, 2, 1),
            lambda: v_mm(0, 3, 0),
            lambda: v_mm(0, 3, 1),
        ))
        attnv_unit(0, 0)
        score_unit(1, 0, (
            lambda: q_mm(1, 0, 0),
            lambda: (q_mm(1, 0, 1), q_front(1, 0, bQ1)),
            lambda: q_mm(1, 1, 0),
            lambda: (q_mm(1, 1, 1), q_front(1, 1, bQ1)),
            lambda: chain(bQ1),
            lambda: q_back2(1, 0),
            lambda: q_back3(1, 0),
        ))
        attnv_unit(0, 1)
        transp_unit(0)
        score_unit(1, 1, (
            lambda: q_back2(1, 1),
            lambda: q_back3(1, 1),
            lambda: k_mm(1, 0, 0),
            lambda: (k_mm(1, 0, 1), k_front(1, 0, bK1a)),
            lambda: k_mm(1, 1, 0),
            lambda: (k_mm(1, 1, 1), k_front(1, 1, bK1a)),
            lambda: chain(bK1a),
            lambda: k_mm(1, 2, 0),
        ))
        attnv_unit(1, 0)
        score_unit(2, 0, (
            lambda: wq_dma(2),
            lambda: (k_mm(1, 2, 1), k_front(1, 2, bK1b)),
            lambda: k_mm(1, 3, 0),
            lambda: (k_mm(1, 3, 1), k_front(1, 3, bK1b)),
            lambda: chain(bK1b),
            lambda: k_back2(1, 0),
            lambda: k_back3(1, 0),
            lambda: k_back2(1, 1),
        ))
        attnv_unit(1, 1)
        transp_unit(1)
        score_unit(2, 1, (
            lambda: k_back3(1, 1),
            lambda: k_back2(1, 2),
            lambda: k_back3(1, 2),
            lambda: k_back2(1, 3),
            lambda: k_back3(1, 3),
            lambda: v_mm(1, 0, 0),
            lambda: v_mm(1, 0, 1),
            lambda: v_mm(1, 1, 0),
        ))
        attnv_unit(2, 0)
        score_unit(3, 0, (
            lambda: q_mm(2, 0, 0),
            lambda: (q_mm(2, 0, 1), q_front(2, 0, bQ2)),
            lambda: q_mm(2, 1, 0),
            lambda: (q_mm(2, 1, 1), q_front(2, 1, bQ2)),
            lambda: chain(bQ2),
            lambda: q_back2(2, 0),
            lambda: q_back3(2, 0),
            lambda: q_back2(2, 1),
        ))
        attnv_unit(2, 1)
        transp_unit(2)
        score_unit(3, 1, (
            lambda: q_back3(2, 1),
            lambda: wq_dma(3),
            lambda: v_mm(1, 1, 1),
            lambda: v_mm(1, 2, 0),
            lambda: v_mm(1, 2, 1),
            lambda: v_mm(1, 3, 0),
            lambda: v_mm(1, 3, 1),
            lambda: pj_dma(0),
        ))
        attnv_unit(3, 0)
        score_unit(4, 0, (
            lambda: q_mm(3, 0, 0),
            lambda: (q_mm(3, 0, 1), q_front(3, 0, bQ3)),
            lambda: q_mm(3, 1, 0),
            lambda: (q_mm(3, 1, 1), q_front(3, 1, bQ3)),
            lambda: chain(bQ3),
            lambda: q_back2(3, 0),
            lambda: q_back3(3, 0),
            lambda: q_back2(3, 1),
        ))
        attnv_unit(3, 1)
        transp_unit(3)
        score_unit(4, 1, (
            lambda: q_back3(3, 1),
            lambda: pj_dma(1),
            lambda: pj_dma(2),
            lambda: pj_dma(3),
        ))
        attnv_unit(4, 0)
        score_unit(5, 0)
        attnv_unit(4, 1)
        transp_unit(4)
        score_unit(5, 1)
        attnv_unit(5, 0)
        score_unit(6, 0)
        attnv_unit(5, 1)
        transp_unit(5)
        score_unit(6, 1, (
            lambda: pa_mm(0, 0, 0),
            lambda: pa_mm(0, 0, 1),
            lambda: pa_mm(0, 1, 0),
            lambda: pa_mm(0, 1, 1),
            lambda: pa_mm(0, 2, 0),
            lambda: pa_mm(0, 2, 1),
            lambda: pa_mm(0, 3, 0),
            lambda: pa_mm(0, 3, 1),
        ))
        attnv_unit(6, 0)
        score_unit(7, 0, (
            lambda: pa_mm(1, 0, 0),
            lambda: pa_mm(1, 0, 1),
            lambda: pa_mm(1, 1, 0),
            lambda: pa_mm(1, 1, 1),
        ))
        attnv_unit(6, 1)
        transp_unit(6)
        score_unit(7, 1, (
            lambda: pa_mm(1, 2, 0),
            lambda: pa_mm(1, 2, 1),
            lambda: pa_mm(1, 3, 0),
            lambda: pa_mm(1, 3, 1),
        ))
        attnv_unit(7, 0)
        attnv_unit(7, 1)
        transp_unit(7)

        # ================= stage 3: projection tail =================
        pt_unit(0, 0)
        pt_unit(1, 0)
        pt_unit(0, 1)
        pt_unit(1, 1)
        pt_unit(0, 2)
        pt_unit(1, 2)
        pt_unit(0, 3)
        pt_unit(1, 3)


_CACHE = {}


def _get_nc():
    if "nc" in _CACHE:
        return _CACHE["nc"]
    nc = bacc.Bacc("TRN2", target_bir_lowering=False, debug=False,
                   enable_asserts=False, num_devices=8)
    bf_shapes = {
        "xfT": (E, N), "wqT": (E, E), "wkT": (E, 256), "wvT": (E, 256),
        "pjT": (E, E), "bpb": (128, 2, 512),
    }
    dr = {k: nc.dram_tensor(k, list(v), BF16, kind="ExternalInput").ap()
          for k, v in bf_shapes.items()}
    bf_shapes2 = {"ckT": (128, N), "skpT": (128, N)}
    for k, v in bf_shapes2.items():
        dr[k] = nc.dram_tensor(k, list(v), BF16, kind="ExternalInput").ap()
    for k, v in {"cqT": (128, R),
                 "sqpT": (128, R), "bq": (128, 8), "bk": (128, 2)}.items():
        dr[k] = nc.dram_tensor(k, list(v), F32, kind="ExternalInput").ap()
    for k, v in {"p2": (128, 128), "bcmask": (128, 128),
                 "summask": (128, 3, 66)}.items():
        dr[k] = nc.dram_tensor(k, list(v), F32R, kind="ExternalInput").ap()
    dr["out"] = nc.dram_tensor("out", [R, E], F32, kind="ExternalOutput").ap()
    with tile.TileContext(nc) as tc:
        _emit(tc, dr)
    nc.compile()
    _CACHE["nc"] = nc
    return nc


def _host_prep(inputs):
    f = np.float32
    import ml_dtypes
    bf = ml_dtypes.bfloat16
    x = np.asarray(inputs["x"], f)
    sin = np.asarray(inputs["sin"], f)
    cos = np.asarray(inputs["cos"], f)
    qn_w = np.asarray(inputs["qn_w"], f)
    kn_w = np.asarray(inputs["kn_w"], f)
    d = np.arange(D)
    sw = d ^ 32
    sign = np.where(d < 32, -1.0, 1.0).astype(f)
    # cos tiles [64, N] rows indexed by d; w folded
    cq64 = (cos * qn_w).T.astype(f)
    ck64 = (cos * kn_w).T.astype(f)
    # permuted sin: sp[e, n] = -sign[e] * w[e] * sin[n, e^32]
    sq64p = (sin.T[sw, :] * (-sign * qn_w)[:, None]).astype(f)
    sk64p = (sin.T[sw, :] * (-sign * kn_w)[:, None]).astype(f)
    cq128 = np.tile(cq64, (2, 1))
    sq128p = np.tile(sq64p, (2, 1))
    ck128 = np.tile(ck64, (2, 1))
    sk128p = np.tile(sk64p, (2, 1))
    p2 = np.zeros((128, 128), f)
    i = np.arange(128)
    p2[i, (i // 64) * 64 + ((i % 64) ^ 32)] = 1.0
    bcm2 = np.zeros((2, 128), f)
    bcm2[0, 0:64] = 1.0
    bcm2[1, 64:128] = 1.0
    bcm128 = np.zeros((128, 128), f)
    for i3 in range(3):
        bcm128[32 * i3:32 * i3 + 2, :] = 8.0 * bcm2
    # 3 zero-padded column-sum masks: variant i sums into out rows 32i:32i+2
    smk = np.zeros((128, 3, 66), f)
    for i3 in range(3):
        smk[:, i3, 32 * i3:32 * i3 + 2] = bcm2.T
    # head permutation: new m index -> old m index
    perm = np.concatenate([np.arange(64 * h, 64 * h + 64) for h in HEAD_ORDER])
    wqT = np.asarray(inputs["wq_w"], f).T   # [e, m]
    pjT = np.asarray(inputs["proj_w"], f).T  # [m, mo]
    bq = np.asarray(inputs["wq_b"], f)
    # v bias folded through proj (softmax rows sum to 1), plus proj bias,
    # broadcast to all 128 partitions
    bv_full = np.asarray(inputs["wv_b"], f).reshape(KV, D)[
        np.arange(H) // (H // KV), :].reshape(E)
    bp_eff = np.asarray(inputs["proj_b"], f) + bv_full @ np.asarray(
        inputs["proj_w"], f).T
    bpb = np.tile(bp_eff.reshape(1, 2, 512), (128, 1, 1)).astype(bf)
    com = {
        "wqT": np.ascontiguousarray(wqT[:, perm]).astype(bf),
        "wkT": np.ascontiguousarray(np.asarray(inputs["wk_w"], f).T).astype(bf),
        "wvT": np.ascontiguousarray(np.asarray(inputs["wv_w"], f).T).astype(bf),
        "pjT": np.ascontiguousarray(pjT[perm, :]).astype(bf),
        "p2": p2, "bcmask": bcm128, "summask": smk,
        "bq": np.ascontiguousarray(bq[perm].reshape(8, 128).T),
        "bk": np.ascontiguousarray(np.asarray(inputs["wk_b"], f).reshape(2, 128).T),
        "bpb": bpb,
    }
    in_maps = []
    for c in range(8):
        b, ch = c // 4, c % 4
        roff = R * ch
        m = dict(com)
        m["xfT"] = np.ascontiguousarray(np.roll(x[b].T, -roff, axis=1)).astype(bf)
        m["ckT"] = np.ascontiguousarray(np.roll(ck128, -roff, axis=1)).astype(bf)
        m["skpT"] = np.ascontiguousarray(np.roll(sk128p, -roff, axis=1)).astype(bf)
        m["cqT"] = np.ascontiguousarray(cq128[:, roff:roff + R])
        m["sqpT"] = np.ascontiguousarray(sq128p[:, roff:roff + R])
        in_maps.append(m)
    return in_maps


def kernel(**inputs):
    nc = _get_nc()
    in_maps = _host_prep(inputs)
    res = bass_utils.run_bass_kernel_spmd(nc, in_maps, core_ids=list(range(8)))
    out = np.empty((B, N, E), np.float32)
    for c in range(8):
        b, ch = c // 4, c % 4
        out[b, R * ch:R * (ch + 1), :] = res.results[c]["out"]
    return out


# revision 24
# speedup vs baseline: 1.2455x; 1.0337x over previous
"""GQA attention kernel for 8 Trainium2 NeuronCores (v3).

Sharding: core c handles batch b = c//4, query rows [512*(c%4), 512*(c%4)+512).
Each core computes K/V for its batch's full (rolled) sequence, all 16 heads of
attention for its 512 query rows, and the final projection. No collectives.

v3 is a scheduling rewrite of v2 driven by TimelineSim engine occupancy:
the Act engine's exp stream (16 units x 8 x [128,1024] ~ 133us) is the hard
floor, and the PE's 154us of matmuls must hide almost entirely under it.

  - QKV/norm/proj work is emitted as <=1us "fill granules" interleaved into
    the score units' w-slots so the PE never runs far ahead or behind the
    exp stream (ps pool double-buffering absorbs one slot of jitter);
  - PSUM re-layout: ps 2x[128,1024] + aux 1x[128,512] (qkv/proj psum) +
    nrm 1x[128,512] (t2p/prb) + po 1x[128,4,65] + pkb 1x[8,512] = 16KB;
  - rsqrt chains are batched: up to 4 subs' sum-of-squares accumulate into
    one [8,512] PSUM tile via zero-padded mask stationaries, so one 5-op
    DVE Newton chain serves 4 norm blocks;
  - v bias folded into the output bias (softmax rows sum to 1 =>
    attn@(v+b) = attn@v + b), proj bias folded into a broadcast SBUF tile
    added during the proj_a PSUM->SBUF copy: zero bias matmuls;
  - startup DMAs split into 512-col chunks ordered along the critical path
    (wk, xk0 -> first k matmul at ~4.5us, first exp at ~13us);
  - projection runs as per-(half,rc) chunks: mt0-5 as fills in units 6-7
    (+ bias via bpb), tail = 2 matmuls per chunk through the freed ps ring.
"""

import numpy as np

import concourse.bass as bass
import concourse.tile as tile
from concourse import bacc, mybir
from concourse import bass_utils

B, N, E = 2, 2048, 1024
H, KV, D = 16, 4, 64
R = 512            # query rows per core
EPS = 1e-6
F32 = mybir.dt.float32
F32R = mybir.dt.float32r
U32 = mybir.dt.uint32
BF16 = mybir.dt.bfloat16
AF = mybir.ActivationFunctionType
ALU = mybir.AluOpType

# head order: tile t holds (HEAD_ORDER[2t] at rows 0:64, HEAD_ORDER[2t+1] at 64:128)
HEAD_ORDER = [0, 4, 1, 5, 2, 6, 3, 7, 8, 12, 9, 13, 10, 14, 11, 15]


def _emit(tc, dr):
    nc = tc.nc
    with (
        tc.tile_pool(name="pers", bufs=1) as pers,
        tc.tile_pool(name="work", bufs=2) as wk,
        tc.tile_pool(name="wqs", bufs=2) as wqs,
        tc.tile_pool(name="ets", bufs=16) as ets,
        tc.tile_pool(name="outs", bufs=2) as outs,
        tc.tile_pool(name="psp", bufs=2, space=bass.MemorySpace.PSUM) as psp,
        tc.tile_pool(name="auxp", bufs=1, space=bass.MemorySpace.PSUM) as auxp,
        tc.tile_pool(name="nrmp", bufs=1, space=bass.MemorySpace.PSUM) as nrmp,
        tc.tile_pool(name="pop", bufs=1, space=bass.MemorySpace.PSUM) as pop,
        tc.tile_pool(name="pkbp", bufs=1, space=bass.MemorySpace.PSUM) as pkbp,
    ):
        # ---------------- persistent tiles ----------------
        kt_til = [pers.tile([128, N], BF16, tag=f"kt{i}", name=f"ktt{i}")
                  for i in range(2)]
        qt_til = [pers.tile([128, 2, R], BF16, tag=f"qt{i}", name=f"qtt{i}")
                  for i in range(4)]
        vt_t = pers.tile([128, 16, 4, 65], BF16, tag="vt")  # v + ones col per g
        ot_t = pers.tile([128, 8, R], BF16, tag="ot")      # attn out (m, q)
        pj_t = pers.tile([128, 8, 1024], BF16, tag="pj")   # proj weights
        p2_t = pers.tile([128, 128], F32R, tag="p2")       # rotate-half perm
        bcm_t = pers.tile([128, 128], F32R, tag="bcm")     # bcast mask (x8 fold)
        smk_t = pers.tile([128, 3, 66], F32R, tag="smk")   # col-sum masks, 3 pads
        bq_t = pers.tile([128, 8], F32, tag="bq")
        bk_t = pers.tile([128, 2], F32, tag="bk")
        bpb_t = pers.tile([128, 2, 512], BF16, tag="bpb")  # proj+v bias bcast
        eps_t = pers.tile([128, 1], F32, tag="eps")
        kmag_t = pers.tile([128, 512], U32, tag="kmag")    # 0x5f3759df

        xk_t = pers.tile([128, 8, N], BF16, tag="xk")
        wk_t = pers.tile([128, 8, 256], BF16, tag="wk")
        wv_t = pers.tile([128, 8, 256], BF16, tag="wv")
        ck_t = pers.tile([128, N], BF16, tag="ck")    # cos*w for K cols
        skp_t = pers.tile([128, N], BF16, tag="skp")  # permuted sign*sin*w for K
        cq_t = pers.tile([128, R], F32, tag="cq")
        sqp_t = pers.tile([128, R], F32, tag="sqp")

        nc.vector.memset(vt_t[:, :, :, 64:65], 1.0)
        nc.vector.memset(kmag_t, 0x5F3759DF)
        nc.vector.memset(eps_t, 64.0 * EPS)

        # ---------- startup DMAs, ordered along the critical path ----------
        xr = dr["xfT"].rearrange("(e p) n -> p e n", p=128)
        nc.sync.dma_start(out=wk_t, in_=dr["wkT"].rearrange("(e p) m -> p e m", p=128))
        nc.sync.dma_start(out=xk_t[:, :, 0:512], in_=xr[:, :, 0:512])
        wq_c0 = wqs.tile([128, 8, 256], BF16, tag="wqc", name="wqc0")
        wqr = dr["wqT"].rearrange("(e p) m -> p e m", p=128)
        nc.sync.dma_start(out=wq_c0, in_=wqr[:, :, 0:256])
        nc.sync.dma_start(out=skp_t[:, 0:1024], in_=dr["skpT"][:, 0:1024])
        nc.sync.dma_start(out=ck_t[:, 0:1024], in_=dr["ckT"][:, 0:1024])
        nc.sync.dma_start(out=bk_t, in_=dr["bk"])
        nc.sync.dma_start(out=bq_t, in_=dr["bq"])
        nc.sync.dma_start(out=cq_t, in_=dr["cqT"])
        nc.sync.dma_start(out=sqp_t, in_=dr["sqpT"])
        nc.sync.dma_start(out=smk_t, in_=dr["summask"])
        nc.sync.dma_start(out=p2_t, in_=dr["p2"])
        nc.sync.dma_start(out=bcm_t, in_=dr["bcmask"])
        nc.sync.dma_start(out=xk_t[:, :, 512:1024], in_=xr[:, :, 512:1024])
        nc.sync.dma_start(out=xk_t[:, :, 1024:1536], in_=xr[:, :, 1024:1536])
        nc.sync.dma_start(out=xk_t[:, :, 1536:2048], in_=xr[:, :, 1536:2048])
        nc.sync.dma_start(out=wv_t, in_=dr["wvT"].rearrange("(e p) m -> p e m", p=128))
        nc.sync.dma_start(out=ck_t[:, 1024:2048], in_=dr["ckT"][:, 1024:2048])
        nc.sync.dma_start(out=skp_t[:, 1024:2048], in_=dr["skpT"][:, 1024:2048])
        nc.sync.dma_start(out=bpb_t, in_=dr["bpb"])

        # ---------------- norm machinery ----------------
        # Per 128-row m-block x 512-token "sub": raw = psum+bias on DVE;
        # squares (gpsimd fast / Act Square slow); sum-of-squares via a
        # zero-padded mask matmul accumulating batch row-pair 2i; rope
        # products u (gpsimd) and t1 (gpsimd); rsqrt for a whole batch in
        # one 5-op DVE Newton chain ([8,512]: free size stays 512); then
        # t2p = P@u, s = t2p+t1, prb = bcast(rsv), out = s*prb.
        class Batch:
            def __init__(self, nm, n_subs, slow):
                self.nm = nm
                self.n = n_subs
                self.slow = slow
                self.next_i = 0
                self.tile = None
                self.rsv = None

        st_u = {}
        st_t1 = {}
        st_s = {}

        def norm_front(key, pr, bias_ap, cs_ap, sp_ap, batch):
            raw = wk.tile([128, 512], F32, tag="raw", bufs=3, name=f"raw{key}")
            nc.vector.tensor_scalar_add(out=raw, in0=pr, scalar1=bias_ap)
            sq = wk.tile([128, 512], F32R, tag="sq", bufs=2, name=f"sq{key}")
            if batch.slow:
                nc.scalar.activation(out=sq, in_=pr, func=AF.Square,
                                     bias=bias_ap)
            else:
                nc.vector.tensor_mul(sq, raw, raw)
            i = batch.next_i
            batch.next_i += 1
            if i == 0:
                batch.tile = pkbp.tile([66, 512], F32, tag="pkb",
                                       name=f"pkb{batch.nm}")
            nc.tensor.matmul(batch.tile, smk_t[:, i, :], sq,
                             start=(i == 0), stop=(i == batch.n - 1))
            u = wk.tile([128, 512], F32R, tag="u", bufs=3, name=f"u{key}")
            nc.gpsimd.tensor_mul(u, raw, sp_ap)
            t1 = wk.tile([128, 512], F32, tag="t1", bufs=3, name=f"t1{key}")
            nc.gpsimd.tensor_mul(t1, raw, cs_ap)
            st_u[key] = u
            st_t1[key] = t1
            return i

        def chain(batch):
            pk = batch.tile
            r = 32 * (batch.n - 1) + 2
            rv = wk.tile([66, 512], F32R, tag="rv", bufs=2,
                         name=f"rv{batch.nm}")
            if batch.slow:
                sdk = wk.tile([66, 512], F32, tag="sdk", bufs=1,
                              name=f"sdk{batch.nm}")
                nc.scalar.activation(out=sdk[0:r], in_=pk[0:r], func=AF.Sqrt,
                                     bias=eps_t[0:r], scale=1.0)
                with nc.allow_low_precision(reason="bf16-level norm"):
                    nc.vector.reciprocal(out=rv[0:r], in_=sdk[0:r])
            else:
                sh = wk.tile([66, 512], U32, tag="sh", bufs=1,
                             name=f"sh{batch.nm}")
                nc.vector.tensor_scalar(out=sh[0:r], in0=pk[0:r].bitcast(U32),
                                        scalar1=1, scalar2=None,
                                        op0=ALU.logical_shift_right)
                y0 = wk.tile([66, 512], U32, tag="y0", bufs=1,
                             name=f"y0{batch.nm}")
                nc.vector.tensor_tensor(out=y0[0:r], in0=kmag_t[0:r],
                                        in1=sh[0:r], op=ALU.subtract)
                y2 = wk.tile([66, 512], F32, tag="y2", bufs=1,
                             name=f"y2{batch.nm}")
                nc.vector.tensor_mul(y2[0:r], y0[0:r].bitcast(F32),
                                     y0[0:r].bitcast(F32))
                nb = wk.tile([66, 512], F32, tag="nb", bufs=1,
                             name=f"nb{batch.nm}")
                nc.vector.scalar_tensor_tensor(out=nb[0:r], in0=pk[0:r],
                                               scalar=-0.5, in1=y2[0:r],
                                               op0=ALU.mult, op1=ALU.mult)
                nc.vector.scalar_tensor_tensor(out=rv[0:r], in0=nb[0:r],
                                               scalar=1.5,
                                               in1=y0[0:r].bitcast(F32),
                                               op0=ALU.add, op1=ALU.mult)
            batch.rsv = rv

        def norm_f2(key):
            u = st_u.pop(key)
            t2p = nrmp.tile([128, 512], F32, tag="nrm", name=f"t2p{key}")
            nc.tensor.matmul(t2p, p2_t, u, start=True, stop=True)
            s = wk.tile([128, 512], F32, tag="s", bufs=2, name=f"s{key}")
            nc.vector.scalar_tensor_tensor(out=s, in0=t2p, scalar=0.0,
                                           in1=st_t1.pop(key),
                                           op0=ALU.add, op1=ALU.add)
            st_s[key] = s

        def norm_f3(key, batch, i, out_ap):
            prb = nrmp.tile([128, 512], F32, tag="nrm", name=f"prb{key}")
            nc.tensor.matmul(prb, bcm_t[32 * i:32 * i + 2, :],
                             batch.rsv[32 * i:32 * i + 2],
                             start=True, stop=True)
            nc.vector.tensor_mul(out_ap, st_s.pop(key), prb)

        # ---------------- k / q / v sub emitters ----------------
        aux_store = {}
        sub_meta = {}

        def k_mm(kt, nb, part):
            key = f"k{kt}{nb}"
            if part == 0:
                aux_store[key] = auxp.tile([128, 512], F32, tag="aux",
                                           name=f"pk{key}")
            pr = aux_store[key]
            for e in range(4 * part, 4 * part + 4):
                nc.tensor.matmul(pr, wk_t[:, e, 128 * kt:128 * (kt + 1)],
                                 xk_t[:, e, 512 * nb:512 * (nb + 1)],
                                 start=(e == 0), stop=(e == 7))

        def k_front(kt, nb, batch):
            key = f"k{kt}{nb}"
            i = norm_front(key, aux_store.pop(key), bk_t[:, kt:kt + 1],
                           ck_t[:, 512 * nb:512 * (nb + 1)],
                           skp_t[:, 512 * nb:512 * (nb + 1)], batch)
            sub_meta[key] = (batch, i)

        def k_back2(kt, nb):
            norm_f2(f"k{kt}{nb}")

        def k_back3(kt, nb):
            key = f"k{kt}{nb}"
            batch, i = sub_meta.pop(key)
            norm_f3(key, batch, i,
                    kt_til[kt][:, 512 * nb:512 * (nb + 1)])

        wq_store = {0: wq_c0}

        def wq_dma(qp):
            wq_c = wqs.tile([128, 8, 256], BF16, tag="wqc", name=f"wqc{qp}")
            nc.sync.dma_start(out=wq_c, in_=wqr[:, :, 256 * qp:256 * (qp + 1)])
            wq_store[qp] = wq_c

        def q_mm(qp, j, part):
            key = f"q{qp}{j}"
            if part == 0:
                aux_store[key] = auxp.tile([128, 512], F32, tag="aux",
                                           name=f"pq{key}")
            pr = aux_store[key]
            wq_c = wq_store[qp]
            for e in range(4 * part, 4 * part + 4):
                nc.tensor.matmul(pr, wq_c[:, e, 128 * j:128 * (j + 1)],
                                 xk_t[:, e, 0:R],
                                 start=(e == 0), stop=(e == 7))

        def q_front(qp, j, batch):
            key = f"q{qp}{j}"
            i = norm_front(key, aux_store.pop(key),
                           bq_t[:, 2 * qp + j:2 * qp + j + 1],
                           cq_t, sqp_t, batch)
            sub_meta[key] = (batch, i)

        def q_back2(qp, j):
            norm_f2(f"q{qp}{j}")

        def q_back3(qp, j):
            key = f"q{qp}{j}"
            batch, i = sub_meta.pop(key)
            norm_f3(key, batch, i, qt_til[qp][:, j, :])

        def v_mm(gp, q4, cpair):
            key = f"v{gp}{q4}"
            if cpair == 0:
                aux_store[key] = auxp.tile([128, 512], F32, tag="aux",
                                           name=f"pv{key}")
            pv = aux_store[key]
            for c in range(2 * cpair, 2 * cpair + 2):
                nch = 4 * q4 + c
                for e in range(8):
                    nc.tensor.matmul(pv[:, 128 * c:128 * (c + 1)],
                                     xk_t[:, e, 128 * nch:128 * (nch + 1)],
                                     wv_t[:, e, 128 * gp:128 * (gp + 1)],
                                     start=(e == 0), stop=(e == 7))
            if cpair == 1:
                pv = aux_store.pop(key)
                nc.vector.tensor_copy(
                    out=vt_t[:, 4 * q4:4 * q4 + 4, 2 * gp:2 * gp + 2, 0:64],
                    in_=pv.rearrange("p (c g x) -> p c g x", c=4, g=2))

        # ---------------- stage-2 unit emitters ----------------
        et_store = {}
        od_store = {}

        def score_unit(t, r01, fills=()):
            ktile = t // 4
            h = HEAD_ORDER[2 * t + r01]
            gq = h // 4
            prow = 64 * (gq % 2)
            assert gq // 2 == ktile and prow == 64 * r01
            qn_h = qt_til[t // 2][prow:prow + 64, t % 2, :]
            etl = []
            fi = 0
            for w in range(8):
                ps = psp.tile([128, 1024], F32, tag="ps", name=f"ps{t}{r01}{w}")
                for c in range(2):
                    nch = 2 * w + c
                    nc.tensor.matmul(
                        ps[:, 512 * c:512 * (c + 1)],
                        kt_til[ktile][prow:prow + 64, 128 * nch:128 * (nch + 1)],
                        qn_h, start=True, stop=True)
                et = ets.tile([128, 1024], BF16, tag="et", bufs=16,
                              name=f"et{t}{r01}{w}")
                etl.append(et)
                nc.scalar.activation(out=et, in_=ps, func=AF.Exp, scale=0.125)
                if fi < len(fills):
                    fills[fi]()
                    fi += 1
            et_store[(t, r01)] = etl
            for f in fills[fi:]:
                f()

        def attnv_unit(t, r01):
            h = HEAD_ORDER[2 * t + r01]
            gq = h // 4
            etl = et_store.pop((t, r01))
            if r01 == 0:
                od_store[t] = outs.tile([128, 4, 128], BF16, tag="od",
                                        name=f"od{t}")
            od = od_store[t]
            po = pop.tile([128, 4, 65], F32, tag="po", name=f"po{t}{r01}")
            for qc in range(4):
                for nch in range(16):
                    nc.tensor.matmul(
                        po[:, qc, :],
                        etl[nch // 2][:, 512 * (nch % 2) + 128 * qc:
                                      512 * (nch % 2) + 128 * (qc + 1)],
                        vt_t[:, nch, gq, :],
                        start=(nch == 0), stop=(nch == 15))
            rcp = outs.tile([128, 4, 1], F32, tag="rcp", name=f"rcp{t}{r01}")
            for qc in range(4):
                nc.vector.reciprocal(out=rcp[:, qc, :],
                                     in_=po[:, qc, 64:65])
                nc.vector.tensor_scalar_mul(
                    out=od[:, qc, 64 * r01:64 * r01 + 64],
                    in0=po[:, qc, 0:64], scalar1=rcp[:, qc, :])

        def transp_unit(t):
            od = od_store.pop(t)
            for qc in range(4):
                nc.sync.dma_start(out=ot_t[:, t, 128 * qc:128 * (qc + 1)],
                                  in_=od[:, qc, :], transpose=True)

        # ---------------- projection ----------------
        pjr = dr["pjT"].rearrange("(m p) e -> p m e", p=128)

        def pj_dma(mp):
            nc.sync.dma_start(out=pj_t[:, 2 * mp:2 * mp + 2, :],
                              in_=pjr[:, 2 * mp:2 * mp + 2, :])

        pf_store = {}
        foA = {}

        def pa_mm(half, rc, g):
            # accumulate mt 3g..3g+2 of proj for output block (half, rc)
            if g == 0:
                pf_store[(half, rc)] = auxp.tile([128, 512], F32, tag="aux",
                                                 name=f"pf{half}{rc}")
            pf = pf_store[(half, rc)]
            for mt in range(3 * g, 3 * g + 3):
                nc.tensor.matmul(pf, ot_t[:, mt, 128 * rc:128 * (rc + 1)],
                                 pj_t[:, mt, 512 * half:512 * (half + 1)],
                                 start=(mt == 0), stop=(mt == 5))
            if g == 1:
                pf = pf_store.pop((half, rc))
                fa = outs.tile([128, 512], BF16, tag="foa", bufs=8,
                               name=f"foa{half}{rc}")
                nc.vector.tensor_tensor(out=fa, in0=pf,
                                        in1=bpb_t[:, half, :], op=ALU.add)
                foA[(half, rc)] = fa

        def pt_unit(half, rc):
            pf2 = psp.tile([128, 512], F32, tag="ps", name=f"pt{half}{rc}")
            for mt in (6, 7):
                nc.tensor.matmul(pf2, ot_t[:, mt, 128 * rc:128 * (rc + 1)],
                                 pj_t[:, mt, 512 * half:512 * (half + 1)],
                                 start=(mt == 6), stop=(mt == 7))
            fo = outs.tile([128, 512], BF16, tag="fo", bufs=4,
                           name=f"fo{half}{rc}")
            nc.vector.scalar_tensor_tensor(
                out=fo, in0=pf2, scalar=0.0, in1=foA.pop((half, rc)),
                op0=ALU.add, op1=ALU.add)
            nc.sync.dma_start(
                out=dr["out"][128 * rc:128 * (rc + 1),
                              512 * half:512 * (half + 1)],
                in_=fo)

        # ================= schedule =================
        bS = Batch("S", 2, slow=True)      # k00, q00 (Act sqrt prologue)
        bFa = Batch("Fa", 1, slow=False)   # k01
        bF = Batch("F", 3, slow=False)     # q01, k02, k03
        bK1a = Batch("K1a", 2, slow=False)  # k10, k11
        bK1b = Batch("K1b", 2, slow=False)  # k12, k13
        bQ1 = Batch("Q1", 2, slow=False)   # q10, q11
        bQ2 = Batch("Q2", 2, slow=False)   # q20, q21
        bQ3 = Batch("Q3", 2, slow=False)   # q30, q31

        # ---- prologue: minimal path to the first exp ----
        # bS (k00+q00, Act sqrt) gates the first exp; k01 (chain bFa) gates
        # w=2; q01/k02/k03 are xk2/3-DMA-gated and chain later (bF).
        k_mm(0, 0, 0); k_mm(0, 0, 1); k_front(0, 0, bS)
        q_mm(0, 0, 0); q_mm(0, 0, 1); q_front(0, 0, bS)
        chain(bS)
        k_back2(0, 0); k_back3(0, 0)
        q_back2(0, 0); q_back3(0, 0)
        k_mm(0, 1, 0); k_mm(0, 1, 1); k_front(0, 1, bFa)
        chain(bFa)
        k_back2(0, 1); k_back3(0, 1)
        q_mm(0, 1, 0); q_mm(0, 1, 1); q_front(0, 1, bF)
        k_mm(0, 2, 0); k_mm(0, 2, 1); k_front(0, 2, bF)
        k_mm(0, 3, 0); k_mm(0, 3, 1); k_front(0, 3, bF)
        chain(bF)
        q_back2(0, 1); q_back3(0, 1)
        k_back2(0, 2); k_back3(0, 2)
        k_back2(0, 3); k_back3(0, 3)

        score_unit(0, 0, (
            lambda: wq_dma(1),
            lambda: v_mm(0, 0, 0),
            lambda: v_mm(0, 0, 1),
            lambda: v_mm(0, 1, 0),
            lambda: v_mm(0, 1, 1),
        ))
        score_unit(0, 1, (
            lambda: v_mm(0, 2, 0),
            lambda: v_mm(0, 2, 1),
            lambda: v_mm(0, 3, 0),
            lambda: v_mm(0, 3, 1),
        ))
        attnv_unit(0, 0)
        score_unit(1, 0, (
            lambda: q_mm(1, 0, 0),
            lambda: (q_mm(1, 0, 1), q_front(1, 0, bQ1)),
            lambda: q_mm(1, 1, 0),
            lambda: (q_mm(1, 1, 1), q_front(1, 1, bQ1)),
            lambda: chain(bQ1),
            lambda: q_back2(1, 0),
            lambda: q_back3(1, 0),
        ))
        attnv_unit(0, 1)
        transp_unit(0)
        score_unit(1, 1, (
            lambda: q_back2(1, 1),
            lambda: q_back3(1, 1),
            lambda: k_mm(1, 0, 0),
            lambda: (k_mm(1, 0, 1), k_front(1, 0, bK1a)),
            lambda: k_mm(1, 1, 0),
            lambda: (k_mm(1, 1, 1), k_front(1, 1, bK1a)),
            lambda: chain(bK1a),
            lambda: k_mm(1, 2, 0),
        ))
        attnv_unit(1, 0)
        score_unit(2, 0, (
            lambda: wq_dma(2),
            lambda: (k_mm(1, 2, 1), k_front(1, 2, bK1b)),
            lambda: k_mm(1, 3, 0),
            lambda: (k_mm(1, 3, 1), k_front(1, 3, bK1b)),
            lambda: chain(bK1b),
            lambda: k_back2(1, 0),
            lambda: k_back3(1, 0),
            lambda: k_back2(1, 1),
        ))
        attnv_unit(1, 1)
        transp_unit(1)
        score_unit(2, 1, (
            lambda: k_back3(1, 1),
            lambda: k_back2(1, 2),
            lambda: k_back3(1, 2),
            lambda: k_back2(1, 3),
            lambda: k_back3(1, 3),
            lambda: v_mm(1, 0, 0),
            lambda: v_mm(1, 0, 1),
            lambda: v_mm(1, 1, 0),
        ))
        attnv_unit(2, 0)
        score_unit(3, 0, (
            lambda: q_mm(2, 0, 0),
            lambda: (q_mm(2, 0, 1), q_front(2, 0, bQ2)),
            lambda: q_mm(2, 1, 0),
            lambda: (q_mm(2, 1, 1), q_front(2, 1, bQ2)),
            lambda: chain(bQ2),
            lambda: q_back2(2, 0),
            lambda: q_back3(2, 0),
            lambda: q_back2(2, 1),
        ))
        attnv_unit(2, 1)
        transp_unit(2)
        score_unit(3, 1, (
            lambda: q_back3(2, 1),
            lambda: wq_dma(3),
            lambda: v_mm(1, 1, 1),
            lambda: v_mm(1, 2, 0),
            lambda: v_mm(1, 2, 1),
            lambda: v_mm(1, 3, 0),
            lambda: v_mm(1, 3, 1),
            lambda: pj_dma(0),
        ))
        attnv_unit(3, 0)
        score_unit(4, 0, (
            lambda: q_mm(3, 0, 0),
            lambda: (q_mm(3, 0, 1), q_front(3, 0, bQ3)),
            lambda: q_mm(3, 1, 0),
            lambda: (q_mm(3, 1, 1), q_front(3, 1, bQ3)),
            lambda: chain(bQ3),
            lambda: q_back2(3, 0),
            lambda: q_back3(3, 0),
            lambda: q_back2(3, 1),
        ))
        attnv_unit(3, 1)
        transp_unit(3)
        score_unit(4, 1, (
            lambda: q_back3(3, 1),
            lambda: pj_dma(1),
            lambda: pj_dma(2),
            lambda: pj_dma(3),
        ))
        attnv_unit(4, 0)
        score_unit(5, 0)
        attnv_unit(4, 1)
        transp_unit(4)
        score_unit(5, 1)
        attnv_unit(5, 0)
        score_unit(6, 0)
        attnv_unit(5, 1)
        transp_unit(5)
        score_unit(6, 1, (
            lambda: pa_mm(0, 0, 0),
            lambda: pa_mm(0, 0, 1),
            lambda: pa_mm(0, 1, 0),
            lambda: pa_mm(0, 1, 1),
            lambda: pa_mm(0, 2, 0),
            lambda: pa_mm(0, 2, 1),
            lambda: pa_mm(0, 3, 0),
            lambda: pa_mm(0, 3, 1),
        ))
        attnv_unit(6, 0)
        score_unit(7, 0, (
            lambda: pa_mm(1, 0, 0),
            lambda: pa_mm(1, 0, 1),
            lambda: pa_mm(1, 1, 0),
            lambda: pa_mm(1, 1, 1),
        ))
        attnv_unit(6, 1)
        transp_unit(6)
        score_unit(7, 1, (
            lambda: pa_mm(1, 2, 0),
            lambda: pa_mm(1, 2, 1),
            lambda: pa_mm(1, 3, 0),
            lambda: pa_mm(1, 3, 1),
        ))
        attnv_unit(7, 0)
        attnv_unit(7, 1)
        transp_unit(7)

        # ================= stage 3: projection tail =================
        pt_unit(0, 0)
        pt_unit(1, 0)
        pt_unit(0, 1)
        pt_unit(1, 1)
        pt_unit(0, 2)
        pt_unit(1, 2)
        pt_unit(0, 3)
        pt_unit(1, 3)


_CACHE = {}


def _get_nc():
    if "nc" in _CACHE:
        return _CACHE["nc"]
    nc = bacc.Bacc("TRN2", target_bir_lowering=False, debug=False,
                   enable_asserts=False, num_devices=8)
    bf_shapes = {
        "xfT": (E, N), "wqT": (E, E), "wkT": (E, 256), "wvT": (E, 256),
        "pjT": (E, E), "bpb": (128, 2, 512),
    }
    dr = {k: nc.dram_tensor(k, list(v), BF16, kind="ExternalInput").ap()
          for k, v in bf_shapes.items()}
    bf_shapes2 = {"ckT": (128, N), "skpT": (128, N)}
    for k, v in bf_shapes2.items():
        dr[k] = nc.dram_tensor(k, list(v), BF16, kind="ExternalInput").ap()
    for k, v in {"cqT": (128, R),
                 "sqpT": (128, R), "bq": (128, 8), "bk": (128, 2)}.items():
        dr[k] = nc.dram_tensor(k, list(v), F32, kind="ExternalInput").ap()
    for k, v in {"p2": (128, 128), "bcmask": (128, 128),
                 "summask": (128, 3, 66)}.items():
        dr[k] = nc.dram_tensor(k, list(v), F32R, kind="ExternalInput").ap()
    dr["out"] = nc.dram_tensor("out", [R, E], BF16, kind="ExternalOutput").ap()
    with tile.TileContext(nc) as tc:
        _emit(tc, dr)
    nc.compile()
    _CACHE["nc"] = nc
    return nc


def _host_prep(inputs):
    f = np.float32
    import ml_dtypes
    bf = ml_dtypes.bfloat16
    x = np.asarray(inputs["x"], f)
    sin = np.asarray(inputs["sin"], f)
    cos = np.asarray(inputs["cos"], f)
    qn_w = np.asarray(inputs["qn_w"], f)
    kn_w = np.asarray(inputs["kn_w"], f)
    d = np.arange(D)
    sw = d ^ 32
    sign = np.where(d < 32, -1.0, 1.0).astype(f)
    # cos tiles [64, N] rows indexed by d; w folded
    cq64 = (cos * qn_w).T.astype(f)
    ck64 = (cos * kn_w).T.astype(f)
    # permuted sin: sp[e, n] = -sign[e] * w[e] * sin[n, e^32]
    sq64p = (sin.T[sw, :] * (-sign * qn_w)[:, None]).astype(f)
    sk64p = (sin.T[sw, :] * (-sign * kn_w)[:, None]).astype(f)
    cq128 = np.tile(cq64, (2, 1))
    sq128p = np.tile(sq64p, (2, 1))
    ck128 = np.tile(ck64, (2, 1))
    sk128p = np.tile(sk64p, (2, 1))
    p2 = np.zeros((128, 128), f)
    i = np.arange(128)
    p2[i, (i // 64) * 64 + ((i % 64) ^ 32)] = 1.0
    bcm2 = np.zeros((2, 128), f)
    bcm2[0, 0:64] = 1.0
    bcm2[1, 64:128] = 1.0
    bcm128 = np.zeros((128, 128), f)
    for i3 in range(3):
        bcm128[32 * i3:32 * i3 + 2, :] = 8.0 * bcm2
    # 3 zero-padded column-sum masks: variant i sums into out rows 32i:32i+2
    smk = np.zeros((128, 3, 66), f)
    for i3 in range(3):
        smk[:, i3, 32 * i3:32 * i3 + 2] = bcm2.T
    # head permutation: new m index -> old m index
    perm = np.concatenate([np.arange(64 * h, 64 * h + 64) for h in HEAD_ORDER])
    wqT = np.asarray(inputs["wq_w"], f).T   # [e, m]
    pjT = np.asarray(inputs["proj_w"], f).T  # [m, mo]
    bq = np.asarray(inputs["wq_b"], f)
    # v bias folded through proj (softmax rows sum to 1), plus proj bias,
    # broadcast to all 128 partitions
    bv_full = np.asarray(inputs["wv_b"], f).reshape(KV, D)[
        np.arange(H) // (H // KV), :].reshape(E)
    bp_eff = np.asarray(inputs["proj_b"], f) + bv_full @ np.asarray(
        inputs["proj_w"], f).T
    bpb = np.tile(bp_eff.reshape(1, 2, 512), (128, 1, 1)).astype(bf)
    com = {
        "wqT": np.ascontiguousarray(wqT[:, perm]).astype(bf),
        "wkT": np.ascontiguousarray(np.asarray(inputs["wk_w"], f).T).astype(bf),
        "wvT": np.ascontiguousarray(np.asarray(inputs["wv_w"], f).T).astype(bf),
        "pjT": np.ascontiguousarray(pjT[perm, :]).astype(bf),
        "p2": p2, "bcmask": bcm128, "summask": smk,
        "bq": np.ascontiguousarray(bq[perm].reshape(8, 128).T),
        "bk": np.ascontiguousarray(np.asarray(inputs["wk_b"], f).reshape(2, 128).T),
        "bpb": bpb,
    }
    in_maps = []
    for c in range(8):
        b, ch = c // 4, c % 4
        roff = R * ch
        m = dict(com)
        m["xfT"] = np.ascontiguousarray(np.roll(x[b].T, -roff, axis=1)).astype(bf)
        m["ckT"] = np.ascontiguousarray(np.roll(ck128, -roff, axis=1)).astype(bf)
        m["skpT"] = np.ascontiguousarray(np.roll(sk128p, -roff, axis=1)).astype(bf)
        m["cqT"] = np.ascontiguousarray(cq128[:, roff:roff + R])
        m["sqpT"] = np.ascontiguousarray(sq128p[:, roff:roff + R])
        in_maps.append(m)
    return in_maps


def kernel(**inputs):
    nc = _get_nc()
    in_maps = _host_prep(inputs)
    res = bass_utils.run_bass_kernel_spmd(nc, in_maps, core_ids=list(range(8)))
    out = np.empty((B, N, E), np.float32)
    for c in range(8):
        b, ch = c // 4, c % 4
        out[b, R * ch:R * (ch + 1), :] = np.asarray(
            res.results[c]["out"], np.float32)
    return out
